# revision 4
# baseline (speedup 1.0000x reference)
"""Trainium2 Bass kernel for nn_Attn_61366492725428 (masked attention pooling).

Reference computation:
    hid = transpose(hidden,(1,0,2)).reshape(B,-1)          # (B, 1024)
    e   = enc @ We + (hid @ Wh)[:,None] + b                # (B, T)
    e   = e * mask
    a   = softmax(e, axis=1) * mask;  a /= a.sum(1)
    ctx = einsum('bt,bth->bh', a, enc)                     # (B, 1024)

Key identities:
  1. The per-batch constant c = hid@Wh + b shifts every *valid* energy
     equally and cancels under the renormalized softmax, so the output
     does not depend on hidden/Wh/b at all:
         ctx[b] = sum_t mask*exp(e_enc)*enc / sum_t mask*exp(e_enc)
  2. Masked positions contribute exactly 0 to both sums, so any
     128-row tile of enc whose mask is all-zero can be skipped
     entirely. Lengths are uniform in [T/4, T], so ~35% of enc never
     needs to leave HBM ("sparse attention").

Structure: the host enumerates valid 128-row tiles ("slots") from the
mask, splits them evenly across the 8 cores (padding with dummy
all-masked slots so every core runs the same fixed-size program), and
packs each core's slots into a contiguous [S,128,1024] array. The
device computes *per-slot partial* results
    ctx_s[h] = sum_t w_t * enc16[t,h],   S_s = sum_t w_t,
    w = exp(e + lmask),  e = sum_h enc16*We16
and the host combines partials per batch: ctx[b] = sum ctx_s / sum S_s
(exact -- same f32 summands, just reassociated).

Device pipeline per slot [128t, 1024h]:
    DMA (SWDGE cast): enc f32 HBM -> fp16 SBUF, U slots per transfer
    DVE : fused mul+reduce  e[t] = sum_h enc16*we16 (f32 accum)
    ACT : w16 = fp16(exp(e + lmask))  (lmask via per-partition bias)
    PE  : ctx_psum[1,1024] += w16^T @ enc16 ; per group: S row via
          ones^T @ w_group
    ACT : PSUM -> SBUF staging;  HWDGE writes staged groups out.

The program depends only on the slot count S (recompiled+cached per S),
so it is correct for any input mask.
"""

import math
import numpy as np

N_CORES = 8
B, T, HE = 32, 2048, 1024
TT = 128                      # t-tile rows (partition dim)
NT = T // TT                  # 16 tiles per batch
NH = 512                      # PSUM bank free-dim limit (f32)
U = 4                         # slots per input DMA (2MB transfers)
G = 8                         # slots per output group
ROW = HE + 1                  # staged row: 1024 ctx values + 1 sum

MODE = "cast"                 # "cast": SWDGE f32->fp16 cast-DMA
                              # "f32dve": HWDGE f32 + DVE product (fallback)
FUSED_REDUCE = False          # DVE fused mul+reduce vs mul + ACT accum

_CACHE = {}


def _build_nc(S):
    import concourse.bacc as bacc
    import concourse.tile as tile
    from concourse import mybir

    f32 = mybir.dt.float32
    f16 = mybir.dt.float16
    Exp = mybir.ActivationFunctionType.Exp
    Copy = mybir.ActivationFunctionType.Copy
    mul_op = mybir.AluOpType.mult
    add_op = mybir.AluOpType.add

    nc = bacc.Bacc("TRN2")
    encp = nc.dram_tensor("encp", [S, TT, HE], f32, kind="ExternalInput")
    lmaskt = nc.dram_tensor("lmaskt", [TT, S], f32, kind="ExternalInput")
    we16b = nc.dram_tensor("we16b", [TT, HE], f16, kind="ExternalInput")
    out = nc.dram_tensor("out", [S, ROW], f32, kind="ExternalOutput")

    n_groups = math.ceil(S / G)

    with tile.TileContext(nc) as tc:
        with (
            tc.tile_pool(name="singles", bufs=1) as singles,
            tc.tile_pool(name="encpool", bufs=3) as encpool,
            tc.tile_pool(name="prodp", bufs=2) as prodp,
            tc.tile_pool(name="stats", bufs=6) as stats,
            tc.tile_pool(name="wgp", bufs=2) as wgp,
            tc.tile_pool(name="stagep", bufs=2) as stagep,
            tc.tile_pool(name="ctxp", bufs=3, space="PSUM") as ctxp,
            tc.tile_pool(name="sp", bufs=2, space="PSUM") as sp,
        ):
            we_sb = singles.tile([TT, HE], f16, tag="we16")
            nc.sync.dma_start(out=we_sb, in_=we16b[:, :])
            mask_all = singles.tile([TT, S], f32, tag="mask")
            nc.sync.dma_start(out=mask_all, in_=lmaskt[:, :])
            ones_col = singles.tile([TT, 1], f16, tag="ones")
            nc.vector.memset(ones_col, 1.0)

            enc_tiles = {}
            for d0 in range(0, S, U):
                du = min(U, S - d0)
                et = encpool.tile([TT, du, HE], f16, tag="enc16")
                nc.gpsimd.dma_start(
                    out=et,
                    in_=encp[d0 : d0 + du].rearrange("u p h -> p u h"),
                )
                for u in range(du):
                    enc_tiles[d0 + u] = et[:, u, :]

            for g0 in range(0, S, G):
                gn = min(G, S - g0)
                stage = stagep.tile([1, gn, ROW], f32, tag="stage")
                w_g = wgp.tile([TT, gn], f16, tag="w_g")
                ctx_list = []
                for k in range(gn):
                    i = g0 + k
                    et = enc_tiles[i]
                    # e[t] = sum_h enc16*we16  (product tensor is a
                    # throwaway; only the f32 accumulator is used)
                    prod = prodp.tile([TT, HE], f16, tag="prod")
                    e_col = stats.tile([TT, 1], f32, tag="e_col")
                    if FUSED_REDUCE:
                        nc.vector.tensor_tensor_reduce(
                            out=prod,
                            in0=et,
                            in1=we_sb,
                            scale=1.0,
                            scalar=0.0,
                            op0=mul_op,
                            op1=add_op,
                            accum_out=e_col,
                        )
                    else:
                        nc.vector.tensor_mul(prod, et, we_sb)
                        nc.scalar.activation(
                            prod, prod, Copy, accum_out=e_col
                        )
                    # w = exp(e + lmask)  (0 valid / -1e4 masked)
                    nc.scalar.activation(
                        w_g[:, k : k + 1], e_col, Exp,
                        bias=mask_all[:, i : i + 1],
                    )
                    # ctx[h] = sum_t w[t] * enc16[t, h]
                    ctx = ctxp.tile([1, 2, NH], f32, tag="ctx")
                    ctx_list.append(ctx)
                    for h in range(2):
                        nc.tensor.matmul(
                            ctx[:, h, :],
                            w_g[:, k : k + 1],
                            et[:, h * NH : (h + 1) * NH],
                            start=True,
                            stop=True,
                        )
                # per-slot sums as one row: s[1, gn] = ones^T @ w_g
                s_ps = sp.tile([1, gn], f32, tag="s_ps")
                nc.tensor.matmul(s_ps, ones_col, w_g, start=True, stop=True)
                nc.scalar.activation(
                    stage[:, :, HE : HE + 1].rearrange("p g o -> p (g o)"),
                    s_ps, Copy,
                )
                for k in range(gn):
                    nc.scalar.activation(
                        stage[:, k, 0:HE].rearrange("p (g h) -> p g h", g=2),
                        ctx_list[k][:, :, :], Copy,
                    )
                nc.scalar.dma_start(
                    out=out[g0 : g0 + gn, :],
                    in_=stage.rearrange("p g r -> p (g r)"),
                )

    nc.compile()
    return nc


def _get_nc(S):
    key = ("nc", MODE, S)
    if key not in _CACHE:
        _CACHE[key] = _build_nc(S)
    return _CACHE[key]


def _plan_slots(mask):
    """Enumerate valid 128-row tiles and split them across cores."""
    valid = mask.reshape(B, NT, TT).max(axis=2) > 0.5     # [B, NT]
    slots = [(b, j) for b in range(B) for j in range(NT) if valid[b, j]]
    if not slots:
        slots = [(0, 0)]
    S = math.ceil(len(slots) / N_CORES)
    per_core = []
    for c in range(N_CORES):
        chunk = slots[c * S : (c + 1) * S]
        per_core.append(chunk + [None] * (S - len(chunk)))
    return per_core, S


def kernel(hidden, encoder_outputs, mask, W, b):
    from concourse import bass_utils

    # avoid S3 upload attempts if tracing is enabled
    bass_utils.upload_artifacts = lambda tmpdir: f"local:{tmpdir}"

    enc = np.asarray(encoder_outputs, dtype=np.float32)
    msk = np.asarray(mask, dtype=np.float32)
    we = np.asarray(W, dtype=np.float32)[0, HE:]          # (1024,)

    per_core, S = _plan_slots(msk)
    nc = _get_nc(S)

    we16b = np.ascontiguousarray(
        np.broadcast_to(we.astype(np.float16), (TT, HE))
    )
    lmask_full = np.where(msk > 0.5, np.float32(0.0), np.float32(-1e4))

    in_maps = []
    for c in range(N_CORES):
        encp = np.zeros((S, TT, HE), dtype=np.float32)
        lm = np.full((S, TT), np.float32(-1e4), dtype=np.float32)
        for i, slot in enumerate(per_core[c]):
            if slot is None:
                continue
            bb, j = slot
            encp[i] = enc[bb, j * TT : (j + 1) * TT, :]
            lm[i] = lmask_full[bb, j * TT : (j + 1) * TT]
        in_maps.append(
            {
                "encp": encp,
                "lmaskt": np.ascontiguousarray(lm.T),
                "we16b": we16b,
            }
        )

    def _run():
        return bass_utils.run_bass_kernel_spmd(
            nc, in_maps, core_ids=list(range(N_CORES))
        )

    try:
        res = _run()
    except Exception:
        # transient device-state failures have been observed; retry once
        res = _run()
    _CACHE["last_results"] = res

    ctx = np.zeros((B, HE), dtype=np.float64)
    ssum = np.zeros(B, dtype=np.float64)
    for c in range(N_CORES):
        rows = res.results[c]["out"]                      # [S, 1025] f32
        for i, slot in enumerate(per_core[c]):
            if slot is None:
                continue
            bb = slot[0]
            ctx[bb] += rows[i, :HE]
            ssum[bb] += rows[i, HE]
    ctx /= ssum[:, None]
    return ctx.astype(np.float32)


# revision 24
# speedup vs baseline: 1.4301x; 1.4301x over previous
"""Trainium2 Bass kernel for nn_Attn_61366492725428 (masked attention pooling).

Reference computation:
    hid = transpose(hidden,(1,0,2)).reshape(B,-1)          # (B, 1024)
    e   = enc @ We + (hid @ Wh)[:,None] + b                # (B, T)
    e   = e * mask
    a   = softmax(e, axis=1) * mask;  a /= a.sum(1)
    ctx = einsum('bt,bth->bh', a, enc)                     # (B, 1024)

Key identities:
  1. The per-batch constant c = hid@Wh + b shifts every *valid* energy
     equally and cancels under the renormalized softmax, so the output
     does not depend on hidden/Wh/b at all:
         ctx[b] = sum_t mask*exp(e_enc)*enc / sum_t mask*exp(e_enc)
  2. Masked positions contribute exactly 0 to both sums, so any
     128-row tile of enc whose mask is all-zero is skipped entirely.
     Lengths are uniform in [T/4, T], so ~35% of enc never needs to
     leave HBM ("sparse attention").

Structure: the host enumerates valid 128-row tiles ("slots") from the
mask, splits them evenly across the 8 cores (padding with dummy
all-masked slots so every core runs the same fixed-size program), and
packs each core's slots into a contiguous [S,128,1024] array. The
device computes *per-slot partial* results
    part_s[h] = sum_t w_t * p16[t,h],   S_s = sum_t w_t,
    p16 = fp16(enc*We),  w = exp(e)*mask,  e = sum_h p16
and the host combines partials per batch (note part_s = We[h] * the
enc-weighted sum, so the host divides by We once):
    ctx[b,h] = sum_s part_s[h] / (sum_s S_s) / We[h]
This is exact reassociation of the same f32 sums.

Device pipeline per slot [128t, 1024h]:
    DMA : HWDGE f32 loads, U slots per transfer (2MB)
    DVE : fused custom op  p16 = fp16(enc*We), e[t] = sum_h enc*We
    ACT : one exp per 4-slot group; DVE masks it: w16 = exp(e)*mask16
    PE  : part_psum[1,1024] += w16^T @ p16, 4 slots sharing one PSUM
          bank pair at partitions {0,32,64,96}; S row via ones^T @ w_g
    ACT : one PSUM->SBUF copy per 4 slots (partition-parallel);
          staged chunks DMA'd out on the ACT HWDGE queue.

The program depends only on the slot count S (recompiled+cached per S),
so it is correct for any input mask.
"""

import math
import numpy as np

N_CORES = 8
B, T, HE = 32, 2048, 1024
TT = 128                      # t-tile rows (partition dim)
NT = T // TT                  # 16 tiles per batch
NH = 512                      # PSUM bank free-dim limit (f32)
U = 4                         # slots per input DMA (2MB transfers)
GS = 2                        # slots accumulated per PSUM partial; the
                              # host pads every batch to an even tile
                              # count so pairs never straddle batches
NSTG = 6                      # pairs per staged output DMA

_CACHE = {}


def _build_nc(S):
    import concourse.bacc as bacc
    import concourse.tile as tile
    from concourse import mybir
    from concourse.dve_ops import TENSOR_TENSOR_REDUCE

    f32 = mybir.dt.float32
    f16 = mybir.dt.float16
    Exp = mybir.ActivationFunctionType.Exp
    Copy = mybir.ActivationFunctionType.Copy

    assert S % GS == 0 and U % GS == 0
    NG = S // GS                      # psum pair groups
    NCHUNK = math.ceil(NG / NSTG)     # output dma chunks

    nc = bacc.Bacc("TRN2")
    encp = nc.dram_tensor("encp", [S, TT, HE], f32, kind="ExternalInput")
    mask16 = nc.dram_tensor("mask16", [TT, S], f16, kind="ExternalInput")
    web = nc.dram_tensor("web", [TT, HE], f32, kind="ExternalInput")
    # out[c, s, :] = ctx partial of slot pair (c*NSTG + s)
    out = nc.dram_tensor(
        "out", [NCHUNK, NSTG, HE], f32, kind="ExternalOutput"
    )
    s_out = nc.dram_tensor("s_out", [1, S], f32, kind="ExternalOutput")

    with tile.TileContext(nc) as tc:
        with (
            tc.tile_pool(name="singles", bufs=1) as singles,
            tc.tile_pool(name="encpool", bufs=3) as encpool,
            tc.tile_pool(name="p16p", bufs=8) as p16p,
            tc.tile_pool(name="stats", bufs=3) as stats,
            tc.tile_pool(name="stagep", bufs=2) as stagep,
            tc.tile_pool(name="ctxp", bufs=3, space="PSUM") as ctxp,
            tc.tile_pool(name="sp", bufs=1, space="PSUM") as sp,
        ):
            we_sb = singles.tile([TT, HE], f32, tag="we")
            nc.sync.dma_start(out=we_sb, in_=web[:, :])
            mask_all = singles.tile([TT, S], f16, tag="mask")
            nc.sync.dma_start(out=mask_all, in_=mask16[:, :])
            ones_col = singles.tile([TT, 1], f16, tag="ones")
            nc.vector.memset(ones_col, 1.0)
            s_stage = singles.tile([1, S], f32, tag="s_stage")
            w_all = singles.tile([TT, S], f16, tag="w_all")

            enc_tiles = {}
            for d0 in range(0, S, U):
                du = min(U, S - d0)
                et = encpool.tile([TT, du, HE], f32, tag="enc")
                nc.sync.dma_start(
                    out=et,
                    in_=encp[d0 : d0 + du].rearrange("u p h -> p u h"),
                )
                for u in range(du):
                    enc_tiles[d0 + u] = et[:, u, :]

            # exp at U-block granularity; psum/copies at GS-pair granularity
            for u0 in range(0, S, U):
                un = min(U, S - u0)
                e_g = stats.tile([TT, un], f32, tag="e_g")
                for k in range(un):
                    p16 = p16p.tile([TT, HE], f16, tag="p16")
                    enc_tiles[u0 + k] = (enc_tiles[u0 + k], p16)
                    # p16 = fp16(enc*We); e[t] = sum_h enc*We (f32)
                    nc.vector._custom_dve(
                        TENSOR_TENSOR_REDUCE,
                        out=p16,
                        in0=enc_tiles[u0 + k][0],
                        in1=we_sb,
                        s0=0.0,
                        s1=1.0,
                        accum_out=e_g[:, k : k + 1],
                    )
                # one exp per U-block; DVE zeroes masked lanes
                ew = stats.tile([TT, un], f16, tag="ew")
                nc.scalar.activation(ew, e_g, Exp)
                nc.vector.tensor_mul(
                    w_all[:, u0 : u0 + un], ew, mask_all[:, u0 : u0 + un]
                )

                for g in range(u0 // GS, (u0 + un) // GS):
                    g0 = g * GS
                    ci, si = divmod(g, NSTG)
                    if si == 0:
                        stage = stagep.tile([1, NSTG, HE], f32, tag="stage")
                    # GS same-batch slots accumulate into one partial
                    ctx2 = ctxp.tile([1, 2, NH], f32, tag="ctx2")
                    for k in range(GS):
                        for h in range(2):
                            nc.tensor.matmul(
                                ctx2[:, h, :],
                                w_all[:, g0 + k : g0 + k + 1],
                                enc_tiles[g0 + k][1][:, h * NH : (h + 1) * NH],
                                start=(k == 0),
                                stop=(k == GS - 1),
                            )
                    nc.scalar.activation(
                        stage[:, si, :].rearrange("p (g h) -> p g h", g=2),
                        ctx2[:, :, :],
                        Copy,
                    )
                    if si == NSTG - 1 or g == NG - 1:
                        nc.scalar.dma_start(
                            out=out[ci][0 : si + 1, :],
                            in_=stage[:, 0 : si + 1, :],
                        )
            # all per-slot sums in one matmul: s[1, S] = ones^T @ w_all
            s_ps = sp.tile([1, S], f32, tag="s_ps")
            nc.tensor.matmul(s_ps, ones_col, w_all, start=True, stop=True)
            nc.scalar.activation(s_stage, s_ps, Copy)
            nc.scalar.dma_start(out=s_out[0:1, :], in_=s_stage)

    nc.compile()
    return nc


def _get_nc(S):
    key = ("nc", S)
    if key not in _CACHE:
        _CACHE[key] = _build_nc(S)
    return _CACHE[key]


def _plan_slots(mask):
    """Enumerate valid 128-row tiles and split them across cores.

    Every batch's tile list is padded to an even count (None = zero
    slot) so that each consecutive pair of slots belongs to a single
    batch -- the device statically accumulates pairs into one partial.
    """
    valid = mask.reshape(B, NT, TT).max(axis=2) > 0.5     # [B, NT]
    slots = []
    for b in range(B):
        tiles = [(b, j) for j in range(NT) if valid[b, j]]
        if len(tiles) % 2:
            tiles.append((b, None))
        slots.extend(tiles)
    if not slots:
        slots = [(0, 0), (0, None)]
    S = math.ceil(len(slots) / N_CORES)
    S = math.ceil(S / GS) * GS
    per_core = []
    for c in range(N_CORES):
        chunk = slots[c * S : (c + 1) * S]
        per_core.append(chunk + [None] * (S - len(chunk)))
    return per_core, S


def kernel(hidden, encoder_outputs, mask, W, b):
    from concourse import bass_utils

    # avoid S3 upload attempts if tracing is enabled
    bass_utils.upload_artifacts = lambda tmpdir: f"local:{tmpdir}"

    enc = np.asarray(encoder_outputs, dtype=np.float32)
    msk = np.asarray(mask, dtype=np.float32)
    we = np.asarray(W, dtype=np.float32)[0, HE:]          # (1024,)

    per_core, S = _plan_slots(msk)
    nc = _get_nc(S)

    web = np.ascontiguousarray(np.broadcast_to(we, (TT, HE)))
    m16_full = (msk > 0.5).astype(np.float16)

    in_maps = []
    for c in range(N_CORES):
        encp = np.zeros((S, TT, HE), dtype=np.float32)
        m16 = np.zeros((S, TT), dtype=np.float16)
        for i, slot in enumerate(per_core[c]):
            if slot is None or slot[1] is None:
                continue
            bb, j = slot
            encp[i] = enc[bb, j * TT : (j + 1) * TT, :]
            m16[i] = m16_full[bb, j * TT : (j + 1) * TT]
        in_maps.append(
            {
                "encp": encp,
                "mask16": np.ascontiguousarray(m16.T),
                "web": web,
            }
        )

    def _run():
        return bass_utils.run_bass_kernel_spmd(
            nc, in_maps, core_ids=list(range(N_CORES))
        )

    try:
        res = _run()
    except Exception:
        # transient device-state failures have been observed; retry once
        res = _run()
    _CACHE["last_results"] = res

    ctx = np.zeros((B, HE), dtype=np.float64)
    ssum = np.zeros(B, dtype=np.float64)
    for c in range(N_CORES):
        rows = res.results[c]["out"]          # [NCHUNK, NSTG, HE]
        svals = res.results[c]["s_out"][0]    # [S]
        for i, slot in enumerate(per_core[c]):
            if slot is None or slot[1] is None:
                continue
            bb = slot[0]
            ssum[bb] += svals[i]
            if i % GS == 0:                   # pair partial, once per pair
                cc, s = divmod(i // GS, NSTG)
                ctx[bb] += rows[cc, s, :]
    ctx /= ssum[:, None]
    ctx /= we.astype(np.float64)[None, :]
    return ctx.astype(np.float32)
